# revision 20
# baseline (speedup 1.0000x reference)
"""Trainium2 Bass kernel for nn_ArchGVAE (GNN message passing VAE loss).

Data-parallel over 8 NeuronCores: 65536 disjoint 4-node/6-edge graphs are
split 8192/core.  The NAS-Bench-201 edge structure is fixed
(src=[0,0,1,0,1,2], dst=[1,2,2,3,3,3]) so all gathers/segment-sums are
static: the host pre-transposes activations to [feature, graph] layout and
the device runs dense matmuls with graphs along the free dimension.

Per-core output is a handful of partial sums (CE pieces + KLD pieces);
the scalar loss is assembled on the host.
"""
import sys
import math

for _p in ("/opt/trn_rl_repo",):
    if _p not in sys.path:
        sys.path.insert(0, _p)

import numpy as np

import concourse.bass as bass
import concourse.tile as tile
from concourse import bacc, mybir
from concourse import bass_utils

F32 = mybir.dt.float32
F32R = mybir.dt.float32r
BF16 = mybir.dt.bfloat16
AF = mybir.ActivationFunctionType
ALU = mybir.AluOpType
AX = mybir.AxisListType

B, NODE, ENUM = 65536, 4, 6
XDIM, EDIM, HDIM, ZDIM = 4, 5, 128, 32
SRC = (0, 0, 1, 0, 1, 2)
DST = (1, 2, 2, 3, 3, 3)
NCORE = 8
G = B // NCORE            # graphs per core
C = 512                   # graphs per chunk
NCH = G // C              # chunks per core
NB = C // 128             # 128-graph blocks per chunk (4)
NEG = -30000.0            # pad value: exp(NEG - x) == 0 in f32, never a max
GW = 3 * ENUM * NB        # label groups per chunk (72), each padded to width 5
CEW = GW * 5              # CE panel width per chunk (360)
ALPHA = 0.01              # leaky-relu slope
EPS_SCALE = 0.01
BETA = 0.005


def _r(ap):
    return ap.bitcast(F32R)


def build(g=G, nch=NCH, stage=4.0):
    """Build + compile the per-core Bass program (SPMD, no collectives)."""
    nc = bacc.Bacc("TRN2", target_bir_lowering=False, debug=False,
                   enable_asserts=False, num_devices=NCORE)

    d_xt = nc.dram_tensor("xt", (NODE * XDIM, g), F32, kind="ExternalInput").ap()
    d_m0 = nc.dram_tensor("m0rhs", (ENUM * 13, g), F32, kind="ExternalInput").ap()
    d_ea = nc.dram_tensor("eat", (ENUM * EDIM, g), F32, kind="ExternalInput").ap()
    d_ar = nc.dram_tensor("arch", (128, (g // 128) * 90), F32, kind="ExternalInput").ap()
    d_ep = nc.dram_tensor("epst", (ZDIM, g), F32, kind="ExternalInput").ap()
    wdefs = {
        "w0": (XDIM, HDIM), "kw0": (13, HDIM),
        "w1": (HDIM, HDIM), "kwd1": (HDIM, HDIM), "kws1": (HDIM, HDIM), "kwe1": (EDIM, HDIM),
        "w2": (HDIM, HDIM), "kwd2": (HDIM, HDIM), "kws2": (HDIM, HDIM), "kwe2": (EDIM, HDIM),
        "fc34": (HDIM, 2 * ZDIM), "fc5": (ZDIM, HDIM),
        "d1": (HDIM, 2 * HDIM), "d2": (HDIM, 180),
    }
    d_w = {k: nc.dram_tensor(k, s, F32, kind="ExternalInput").ap() for k, s in wdefs.items()}
    d_out = nc.dram_tensor("out", (128, 8), F32, kind="ExternalOutput").ap()

    with tile.TileContext(nc) as tc:
        with (
            tc.tile_pool(name="wts", bufs=1) as pw,
            tc.tile_pool(name="acc", bufs=1) as pacc,
            tc.tile_pool(name="pin", bufs=2) as pin,
            tc.tile_pool(name="hb", bufs=9) as ph,
            tc.tile_pool(name="msb", bufs=3) as pms,
            tc.tile_pool(name="dec", bufs=2) as pdec,
            tc.tile_pool(name="pm", bufs=2, space="PSUM") as ppm,   # 3-bank slots
            tc.tile_pool(name="pr", bufs=2, space="PSUM") as ppr,   # 1-bank slots
        ):
            # ---- persistent weights ----
            w = {}
            for k, shape in wdefs.items():
                dt_k = F32 if k == "d2" else F32R
                w[k] = pw.tile(list(shape), dt_k, name=f"w_{k}")
                nc.sync.dma_start(w[k][:], d_w[k].bitcast(dt_k))
            d2bf = pw.tile([HDIM, 180], BF16, name="d2bf")
            nc.scalar.activation(d2bf[:], w["d2"][:], AF.Copy)
            lneps = pw.tile([ZDIM, 1], F32, name="lneps")
            nc.gpsimd.memset(lneps[:], float(math.log(EPS_SCALE)))

            # ---- persistent accumulators ----
            sexp_all = pacc.tile([128, GW * nch], F32, name="sexp_all")
            acc_nolog = pacc.tile([128, nch], F32, name="acc_nolog")
            acc_kld = pacc.tile([ZDIM, 3 * nch], F32, name="acc_kld")
            ot = pacc.tile([128, 8], F32, name="ot")
            nc.vector.memset(ot[:], 0.0)
            if stage < 4:
                nc.vector.memset(sexp_all[:], 1.0)
                nc.vector.memset(acc_nolog[:], 0.0)
                nc.vector.memset(acc_kld[:], 0.0)

            for c in range(nch):
                cs = slice(c * C, (c + 1) * C)

                xt_t = []
                for n in range(NODE):
                    t = pin.tile([XDIM, C], F32R, name=f"xt_{c}_{n}", tag=f"xt{n}")
                    nc.sync.dma_start(t[:], d_xt[XDIM * n:XDIM * n + XDIM, cs].bitcast(F32R))
                    xt_t.append(t)
                m0_t = []
                for j in range(ENUM):
                    t = pin.tile([13, C], F32R, name=f"m0_{c}_{j}", tag=f"m0{j}")
                    nc.sync.dma_start(t[:], d_m0[13 * j:13 * j + 13, cs].bitcast(F32R))
                    m0_t.append(t)
                ea_t = []
                for j in range(ENUM):
                    t = pin.tile([EDIM, C], F32R, name=f"ea_{c}_{j}", tag=f"ea{j}")
                    nc.sync.dma_start(t[:], d_ea[EDIM * j:EDIM * j + EDIM, cs].bitcast(F32R))
                    ea_t.append(t)
                ar_t = pin.tile([128, CEW], F32, name="ar_t", tag="ar")
                nc.sync.dma_start(ar_t[:], d_ar[:, c * CEW:(c + 1) * CEW])
                ep_t = pin.tile([ZDIM, C], F32, name="ep_t", tag="ep")
                nc.sync.dma_start(ep_t[:], d_ep[:, cs])

                # ================= conv layers =================
                h = [None] * NODE
                for L in range(3):
                    mA = ppm.tile([128, 3 * C], F32, name=f"mA{L}_{c}", tag="pm")
                    mB = ppm.tile([128, 3 * C], F32, name=f"mB{L}_{c}", tag="pm")
                    for j in range(ENUM):
                        dst_ps = (mA if j < 3 else mB)[:, (j % 3) * C:(j % 3) * C + C]
                        if L == 0:
                            nc.tensor.matmul(dst_ps, _r(w["kw0"][:]),
                                             _r(m0_t[j][:]),
                                             start=True, stop=True)
                        else:
                            kd, ks, ke = w[f"kwd{L}"], w[f"kws{L}"], w[f"kwe{L}"]
                            nc.tensor.matmul(dst_ps, _r(kd[:]), _r(h[DST[j]][:]),
                                             start=True, stop=False)
                            nc.tensor.matmul(dst_ps, _r(ks[:]), _r(h[SRC[j]][:]),
                                             start=False, stop=False)
                            nc.tensor.matmul(dst_ps, _r(ke[:]),
                                             _r(ea_t[j][:]),
                                             start=False, stop=True)
                    mAs = pms.tile([128, 3 * C], F32, name=f"mAs{L}_{c}", tag="ms")
                    nc.scalar.activation(mAs[:], mA[:], AF.Prelu, alpha=ALPHA)
                    mBs = pms.tile([128, 3 * C], F32, name=f"mBs{L}_{c}", tag="ms")
                    nc.scalar.activation(mBs[:], mB[:], AF.Prelu, alpha=ALPHA)

                    wl = w[f"w{L}" if L else "w0"]
                    def _rhs(n):
                        if L == 0:
                            return _r(xt_t[n][:])
                        return _r(h[n][:])
                    r01 = ppr.tile([128, C], F32, name=f"r01_{L}_{c}", tag="pr")
                    r1 = ppr.tile([128, C], F32, name=f"r1_{L}_{c}", tag="pr")
                    nc.tensor.matmul(r01[:], _r(wl[:]), _rhs(0), start=True, stop=True)
                    nc.tensor.matmul(r1[:], _r(wl[:]), _rhs(1), start=True, stop=True)

                    hn = [ph.tile([128, C], F32R, name=f"h{L}_{c}_{n}", tag="h")
                          for n in range(NODE)]
                    # node0: plain copy of resid; node1: resid + m0
                    nc.vector.tensor_copy(hn[0][:], r01[:])
                    nc.vector.tensor_add(hn[1][:], r1[:], mAs[:, 0:C])
                    r2 = ppr.tile([128, C], F32, name=f"r2_{L}_{c}", tag="pr")
                    r3 = ppr.tile([128, C], F32, name=f"r3_{L}_{c}", tag="pr")
                    nc.tensor.matmul(r2[:], _r(wl[:]), _rhs(2), start=True, stop=True)
                    nc.tensor.matmul(r3[:], _r(wl[:]), _rhs(3), start=True, stop=True)
                    # node2: resid + (m1+m2);  s12 computed on gpsimd in-place
                    nc.gpsimd.tensor_add(mAs[:, C:2 * C], mAs[:, C:2 * C], mAs[:, 2 * C:3 * C])
                    nc.vector.tensor_add(hn[2][:], r2[:], mAs[:, C:2 * C])
                    # node3: resid + (m3+m4+m5)
                    nc.gpsimd.tensor_add(mBs[:, 0:C], mBs[:, 0:C], mBs[:, C:2 * C])
                    nc.vector.tensor_add(hn[3][:], r3[:], mBs[:, 2 * C:3 * C])
                    nc.gpsimd.tensor_add(hn[3][:], hn[3][:], mBs[:, 0:C])
                    h = hn

                if stage < 2:
                    for n in range(NODE):
                        nc.vector.reduce_sum(ot[:, 5 + (n % 2):6 + (n % 2)], h[n][:].bitcast(F32), axis=AX.X)
                    continue
                # ================= VAE head =================
                muv = ppr.tile([64, C], F32, name=f"muv_{c}", tag="pr")
                for n in range(NODE):
                    nc.tensor.matmul(muv[:], _r(w["fc34"][:]), _r(h[n][:]),
                                     start=(n == 0), stop=(n == NODE - 1))
                mu, lv = muv[0:ZDIM, :], muv[ZDIM:64, :]
                # sfac = eps_scale * exp(0.5 * lv)
                sfac = pdec.tile([ZDIM, C], F32, name=f"sfac_{c}", tag="sfac")
                nc.scalar.activation(sfac[:], lv, AF.Exp, scale=0.5,
                                     bias=lneps[:])
                zt = pdec.tile([ZDIM, C], F32R, name=f"zt_{c}", tag="zt")
                nc.gpsimd.tensor_mul(zt[:], ep_t[:], sfac[:])
                nc.vector.tensor_add(zt[:], zt[:], mu)
                # KLD partials via ACT accum_out only (no DVE):
                # cols [c]=sum(mu^2), [nch+c]=sum(exp lv), [2nch+c]=sum(lv)
                sq = pdec.tile([ZDIM, C], F32, name=f"sq_{c}", tag="sq")
                nc.scalar.activation(sq[:], mu, AF.Square,
                                     accum_out=acc_kld[:, c:c + 1])
                nc.scalar.activation(sq[:], lv, AF.Exp,
                                     accum_out=acc_kld[:, nch + c:nch + c + 1])
                nc.scalar.activation(sq[:], lv, AF.Identity,
                                     accum_out=acc_kld[:, 2 * nch + c:2 * nch + c + 1])

                if stage < 3:
                    nc.vector.reduce_sum(ot[:, 5:6], sfac[:].broadcast_to((128, C)) if False else sq[:, 0:C].partition_broadcast(128) if False else zt[:].bitcast(F32), axis=AX.X) if False else None
                    nc.vector.reduce_sum(ot[0:ZDIM, 5:6], zt[:].bitcast(F32), axis=AX.X)
                    continue
                hgp = ppr.tile([128, C], F32, name=f"hgp_{c}", tag="pr")
                nc.tensor.matmul(hgp[:], _r(w["fc5"][:]), _r(zt[:]), start=True, stop=True)
                Hg = pdec.tile([128, C], F32R, name=f"Hg_{c}", tag="Hg")
                nc.scalar.activation(Hg[:], hgp[:], AF.Tanh)

                d1a = ppr.tile([128, C], F32, name=f"d1a_{c}", tag="pr")
                d1b = ppr.tile([128, C], F32, name=f"d1b_{c}", tag="pr")
                nc.tensor.matmul(d1a[:], _r(w["d1"][:, 0:HDIM]), _r(Hg[:]), start=True, stop=True)
                nc.tensor.matmul(d1b[:], _r(w["d1"][:, HDIM:2 * HDIM]), _r(Hg[:]), start=True, stop=True)
                ta = pdec.tile([128, C], F32, name=f"ta_{c}", tag="ta")
                nc.scalar.activation(ta[:], d1a[:], AF.Prelu, alpha=ALPHA)
                h1 = pdec.tile([128, C], BF16, name=f"h1_{c}", tag="h1")
                nc.vector.tensor_add(h1[:], ta[:], d1b[:])

                # d2 (role-swap): stationary = h1 block, moving = weights.
                # psum block k at column offset 256*k holds [mw(90) | rw(90)].
                d2p = ppm.tile([128, 4 * 256], F32, name=f"d2p_{c}", tag="pm")
                for k in range(NB):
                    nc.tensor.matmul(d2p[:, 256 * k:256 * k + 180],
                                     h1[:, 128 * k:128 * k + 128],
                                     d2bf[:], start=True, stop=True)
                pred = pdec.tile([128, CEW], F32, name=f"pred_{c}", tag="pred")
                # Evict mw part with leaky-relu, then add rw part. The d2
                # weights carry zero pad columns, so pad positions come out 0;
                # re-stamp them to NEG afterwards (exp(NEG-x)==0, never a max).
                psrc_mw = d2p[:].rearrange("p (k v) -> p k v", k=4)[:, :, 0:90]
                psrc_rw = d2p[:].rearrange("p (k v) -> p k v", k=4)[:, :, 90:180]
                pdst = pred[:].rearrange("p (k v) -> p k v", k=4)
                nc.scalar.activation(pdst, psrc_mw, AF.Prelu, alpha=ALPHA)
                nc.vector.tensor_add(pdst, pdst, psrc_rw)
                pad0 = pred[:].rearrange("p (k j i) -> p k j i", k=4, j=6)[:, :, :, 4:5]
                pad1 = pred[:].rearrange("p (k j i) -> p k j i", k=4, j=6)[:, :, :, 9:10]
                nc.gpsimd.memset(pad0, NEG)
                nc.gpsimd.memset(pad1, NEG)

                if stage < 3.5:
                    nc.vector.reduce_sum(ot[:, 5:6], pred[:], axis=AX.X)
                    continue
                # ================= CE =================
                a3 = ar_t[:].rearrange("p (q i) -> p q i", i=5)
                p3 = pred[:].rearrange("p (q i) -> p q i", i=5)
                amax = pdec.tile([128, GW], F32, name=f"amax_{c}", tag="amax")
                lmax = pdec.tile([128, GW], F32, name=f"lmax_{c}", tag="lmax")
                nc.vector.reduce_max(amax[:], a3, axis=AX.X)
                nc.vector.reduce_max(lmax[:], p3, axis=AX.X)
                mask = pdec.tile([128, CEW], F32, name=f"mask_{c}", tag="mask")
                m3 = mask[:].rearrange("p (q i) -> p q i", i=5)
                amax_b = amax[:].broadcast_to((128, GW, 5))
                lmax_b = lmax[:].broadcast_to((128, GW, 5))
                if stage < 3.6:
                    nc.vector.reduce_sum(ot[:, 5:6], amax[:], axis=AX.X)
                    continue
                nc.vector.tensor_tensor(m3, a3, amax_b, op=ALU.is_equal)
                nc.vector.tensor_mul(m3, m3, p3)
                pick = pdec.tile([128, GW], F32, name=f"pick_{c}", tag="pick")
                nc.vector.reduce_sum(pick[:], m3, axis=AX.X)
                if stage < 3.8:
                    nc.vector.reduce_sum(ot[:, 5:6], pick[:], axis=AX.X)
                    continue
                eb = pdec.tile([128, CEW], F32, name=f"eb_{c}", tag="eb")
                e3 = eb[:].rearrange("p (q i) -> p q i", i=5)
                nc.vector.tensor_sub(e3, p3, lmax_b)
                nc.scalar.activation(eb[:], eb[:], AF.Exp)
                nc.vector.reduce_sum(sexp_all[:, c * GW:(c + 1) * GW], e3, axis=AX.X)
                if stage < 3.9:
                    continue
                junk = pdec.tile([128, GW], F32, name=f"junk_{c}", tag="junk")
                nc.vector.tensor_sub(junk[:], lmax[:], pick[:])
                nc.vector.reduce_sum(acc_nolog[:, c:c + 1], junk[:], axis=AX.X)

            # ---- final: deferred logs + reductions ----
            lnb = pacc.tile([128, GW * nch], F32, name="lnb")
            nc.scalar.activation(lnb[:], sexp_all[:], AF.Ln)
            nc.vector.reduce_sum(ot[:, 0:1], lnb[:], axis=AX.X)
            nc.vector.reduce_sum(ot[:, 1:2], acc_nolog[:], axis=AX.X)
            nc.vector.reduce_sum(ot[0:ZDIM, 2:3], acc_kld[:, 0:nch], axis=AX.X)
            nc.vector.reduce_sum(ot[0:ZDIM, 3:4], acc_kld[:, nch:2 * nch], axis=AX.X)
            nc.vector.reduce_sum(ot[0:ZDIM, 4:5], acc_kld[:, 2 * nch:3 * nch], axis=AX.X)
            nc.sync.dma_start(d_out, ot[:])

    nc.compile()
    return nc


def _pack_host(inputs, g=G, nch=NCH):
    """Shard + lay out inputs for the device program. Returns in_maps list."""
    f32 = np.float32
    x = np.ascontiguousarray(inputs["x"], dtype=f32).reshape(NCORE, g, NODE, XDIM)
    ea = np.ascontiguousarray(inputs["edge_attr"], dtype=f32).reshape(NCORE, g, ENUM, EDIM)
    arch = np.ascontiguousarray(inputs["arch_tensor"], dtype=f32).reshape(NCORE, g, ENUM, 13)
    eps = np.ascontiguousarray(inputs["eps"], dtype=f32).reshape(NCORE, g, ZDIM)

    xt = x.transpose(0, 2, 3, 1).reshape(NCORE, NODE * XDIM, g)
    eat = ea.transpose(0, 2, 3, 1).reshape(NCORE, ENUM * EDIM, g)
    epst = eps.transpose(0, 2, 1)
    m0rhs = np.concatenate(
        [np.concatenate([x[:, :, DST[j], :], x[:, :, SRC[j], :], ea[:, :, j, :]], axis=2)
         for j in range(ENUM)], axis=2).transpose(0, 2, 1)  # (NCORE, 78, g)

    # CE panel: per chunk, groups (k-block, slot, grp) each padded to width 5.
    nblk = g // 128
    ar = np.full((NCORE, nch, NB, 128, ENUM, 3, 5), NEG, dtype=f32)
    a6 = arch.reshape(NCORE, nch, NB, 128, ENUM, 13)
    ar[..., 0, 0:4] = a6[..., 0:4]
    ar[..., 1, 0:4] = a6[..., 4:8]
    ar[..., 2, 0:5] = a6[..., 8:13]
    # -> (NCORE, 128, nch * NB*ENUM*3*5) with k-block-major inside chunk
    ar = ar.transpose(0, 3, 1, 2, 4, 5, 6).reshape(NCORE, 128, nch * NB * ENUM * 3 * 5)
    del a6

    # weights (fold once, float64 for the products)
    def W(k):
        return np.asarray(inputs[k], np.float64)
    for bname in ("c0_rb1", "c0_rb2", "c1_rb1", "c1_rb2", "c2_rb1", "c2_rb2",
                  "fc3_b", "fc4_b", "fc5_b", "d1_mb", "d1_rb", "d2_mb", "d2_rb"):
        assert not np.any(np.asarray(inputs[bname])), f"nonzero bias {bname} unsupported"

    wts = {
        "w0": (W("c0_rw1") @ W("c0_rw2")).astype(f32),
        "kw0": np.asarray(inputs["c0_kw"], f32),
        "w1": (W("c1_rw1") @ W("c1_rw2")).astype(f32),
        "w2": (W("c2_rw1") @ W("c2_rw2")).astype(f32),
        "fc34": np.concatenate([inputs["fc3_w"], inputs["fc4_w"]], axis=1).astype(f32),
        "fc5": np.asarray(inputs["fc5_w"], f32),
        "d1": np.concatenate([inputs["d1_mw"], inputs["d1_rw"]], axis=1).astype(f32),
    }
    for L in (1, 2):
        kw = np.asarray(inputs[f"c{L}_kw"], f32)
        wts[f"kwd{L}"] = np.ascontiguousarray(kw[0:HDIM])
        wts[f"kws{L}"] = np.ascontiguousarray(kw[HDIM:2 * HDIM])
        wts[f"kwe{L}"] = np.ascontiguousarray(kw[2 * HDIM:2 * HDIM + EDIM])
    # d2, padded to the CE panel layout: per slot [in(4)+pad, out(4)+pad, et(5)]
    d2m = np.asarray(inputs["d2_mw"], f32)   # (128, 78)
    d2r = np.asarray(inputs["d2_rw"], f32)
    d2 = np.zeros((HDIM, 180), dtype=f32)
    for j in range(ENUM):
        for part, src_np in ((0, d2m), (90, d2r)):
            blk = src_np[:, 13 * j:13 * j + 13]
            d2[:, part + 15 * j + 0: part + 15 * j + 4] = blk[:, 0:4]
            d2[:, part + 15 * j + 5: part + 15 * j + 9] = blk[:, 4:8]
            d2[:, part + 15 * j + 10:part + 15 * j + 15] = blk[:, 8:13]
    wts["d2"] = d2

    in_maps = []
    for core in range(NCORE):
        m = {
            "xt": np.ascontiguousarray(xt[core]),
            "m0rhs": np.ascontiguousarray(m0rhs[core]),
            "eat": np.ascontiguousarray(eat[core]),
            "arch": np.ascontiguousarray(ar[core]),
            "epst": np.ascontiguousarray(epst[core]),
        }
        m.update(wts)
        in_maps.append(m)
    return in_maps


def _combine_host(outs):
    """outs: list of per-core 'out' arrays (128,4). Returns scalar loss."""
    ce = 0.0
    mu2 = 0.0
    lvt = 0.0
    for o in outs:
        o = np.asarray(o, np.float64)
        ce += o[:, 0].sum() + o[:, 1].sum()
        mu2 += o[0:ZDIM, 2].sum()
        lvt += o[0:ZDIM, 4].sum() - o[0:ZDIM, 3].sum()
    res = ce / (B * ENUM)
    kld_inner = (B * ZDIM) + lvt - mu2
    kld = -0.5 * kld_inner / (B * ZDIM)
    return np.float32(res + BETA * kld)


_NC_CACHE = {}


def _get_nc():
    if "nc" not in _NC_CACHE:
        _NC_CACHE["nc"] = build()
    return _NC_CACHE["nc"]


def kernel(**inputs):
    nc = _get_nc()
    in_maps = _pack_host(inputs)
    res = bass_utils.run_bass_kernel_spmd(nc, in_maps, core_ids=list(range(NCORE)))
    outs = [r["out"] for r in res.results]
    return np.array(_combine_host(outs), dtype=np.float32)


if __name__ == "__main__":
    import reference as R
    inp = {k: np.asarray(v) for k, v in R.setup_inputs().items()}
    got = kernel(**inp)
    exp = np.asarray(R.reference(**R.setup_inputs()))
    rel = abs(float(got) - float(exp)) / abs(float(exp))
    print(f"expected={float(exp):.6f} got={float(got):.6f} rel={rel:.3e}")
